# revision 1
# baseline (speedup 1.0000x reference)
"""Bass/TRN2 kernel for nn_CustomLoss_46024869544057.

Computes: BCE loss mean * (1 + 0.1 * count(p > 0.5 & t == 0)) over N=2^24
elements, data-parallel across 8 NeuronCores.

Per-core math (shard of 2^21 elements viewed as [128, 16384]):
  s2 = t - 0.5                      (DVE: tensor_scalar, int32 -> f32)
  y2 = (p - 0.5) * s2               (DVE: fused scalar_tensor_tensor)
  q  = 2*y2 + 0.5 = t ? p : 1-p     (folded into Ln's scale/bias)
  ln(q) summed per partition        (ACT: Ln with accum_out)
  c  = (p - 1) > s2                 (DVE: fused STT, == (t==0 & p>0.5),
                                     counted per partition via accum_out)
Host: sum the per-(partition, tile) partials in f64, finish
  -(lnsum/N) * (1 + 0.1*count).
"""

import sys

for _p in ("/opt/trn_rl_repo",):
    if _p not in sys.path:
        sys.path.insert(0, _p)

from contextlib import ExitStack

import numpy as np

import concourse.bass as bass
import concourse.tile as tile
from concourse import bacc
from concourse import mybir
from concourse.alu_op_type import AluOpType
from concourse.bass_utils import run_bass_kernel_spmd

N = 16_777_216
NCORES = 8
PER = N // NCORES  # 2_097_152
P = 128
FREE = PER // P  # 16384
# Ramped tile sizes: small leading tiles shrink the pipeline-fill latency
# (a 512-col DMA lands in ~1.7us vs ~6.2us for 2048 cols), and small
# trailing tiles shrink the drain latency (the last tile's serial
# DMA->s2->y2/c->Ln chain sits entirely after the last bulk DMA).  The
# steady state runs on 2048-col tiles.  Sum must equal FREE.
SIZES = [512, 512, 1024, 2048, 2048, 2048, 2048, 2048, 2048, 1024, 512, 512]
assert sum(SIZES) == FREE
NTILES = len(SIZES)

# Exposed for test harnesses: the BassKernelResults of the last kernel() call.
last_results = None


def _build():
    # Keep GpSimd instruction-free: Bass.__init__ emits its const-AP memsets
    # on the Pool engine, which costs a ~2.7us Q7 launch in the preamble and
    # a ~3.5us Q7 library-load/drain in the tail.  Redirect those memsets to
    # DVE for the duration of construction.
    # Also skip the framework's preamble all_engine_barrier: it stalls ~4-6us
    # (gated on the Tensor engine booting, which this kernel never uses) and
    # only orders the const-AP memsets, which nothing here depends on --
    # Tile tracks the one const we do use (half) through its own dep graph.
    orig_memset = bass.BassGpSimd.memset
    orig_barrier = bass.Bass.all_engine_barrier
    bass.BassGpSimd.memset = lambda self, ap, c: self.bass.vector.memset(ap, c)
    bass.Bass.all_engine_barrier = lambda self, *a, **k: None
    try:
        nc = bacc.Bacc("TRN2", target_bir_lowering=False, debug=False)
    finally:
        bass.BassGpSimd.memset = orig_memset
        bass.Bass.all_engine_barrier = orig_barrier
    p_dram = nc.dram_tensor("inputs", [P, FREE], mybir.dt.float32, kind="ExternalInput").ap()
    t_dram = nc.dram_tensor("targets", [P, FREE], mybir.dt.int32, kind="ExternalInput").ap()
    out_dram = nc.dram_tensor(
        "partials", [P, 2 * NTILES], mybir.dt.float32, kind="ExternalOutput"
    ).ap()

    with tile.TileContext(nc) as tc, ExitStack() as ctx:
        io_pool = ctx.enter_context(tc.tile_pool(name="io", bufs=4))
        work_pool = ctx.enter_context(tc.tile_pool(name="work", bufs=3))
        out_sc = ctx.enter_context(tc.tile_pool(name="out_sc", bufs=2))
        acc_pool = ctx.enter_context(tc.tile_pool(name="acc", bufs=1))
        acc_cnt = acc_pool.tile([P, NTILES], mybir.dt.float32, tag="acc_cnt")
        acc_ln = acc_pool.tile([P, NTILES], mybir.dt.float32, tag="acc_ln")
        half = acc_pool.tile([P, 1], mybir.dt.float32, tag="half")
        nc.vector.memset(half[:], 0.5)
        # Warm the ACT function tables (Copy + Ln) on 1-column dummies so the
        # ~1.3us table-load DMAs happen during the first input transfers, not
        # in the middle of the pipeline.
        warm = acc_pool.tile([P, 1], mybir.dt.float32, tag="warm")
        nc.scalar.activation(
            warm[:], half[:], mybir.ActivationFunctionType.Ln, bias=half[:], scale=1.0
        )
        # Engine split: ACT converts targets (s2) and does the Ln pass, DVE
        # does the two fused STT passes, DMA (~47us) is the roofline.  The
        # ACT stream is software-pipelined one stage ahead (s2(i) is emitted
        # before Ln(i-1)) so the s2->y2->Ln chain doesn't serialize a tile:
        # each engine works on a different tile concurrently.
        MAXF = max(SIZES)
        offs = [sum(SIZES[:i]) for i in range(NTILES)]
        pend = None  # (y2_tile, size, index) awaiting its Ln pass

        def emit_ln(p):
            y2p, fp, ip = p
            lnout = out_sc.tile([P, MAXF], mybir.dt.float32, tag="ln")
            nc.scalar.activation(
                lnout[:, :fp], y2p[:, :fp], mybir.ActivationFunctionType.Ln,
                bias=half[:], scale=2.0,
                accum_out=acc_ln[:, ip : ip + 1],
            )

        for i in range(NTILES):
            f, off = SIZES[i], offs[i]
            pt = io_pool.tile([P, MAXF], mybir.dt.float32, tag="p")
            tt = io_pool.tile([P, MAXF], mybir.dt.int32, tag="t")
            nc.sync.dma_start(tt[:, :f], t_dram[:, off : off + f])
            nc.sync.dma_start(pt[:, :f], p_dram[:, off : off + f])
            # s2 = t - 0.5 (int32 -> f32) on DVE; single-tensor-operand
            # tensor_scalar runs in the 2x perf mode, so all three DVE
            # passes (~5.5us/2048-col tile) still fit under the ~6.5us DMA
            # period, and the cross-engine s2 handoff disappears.
            s2 = work_pool.tile([P, MAXF], mybir.dt.float32, tag="s2")
            nc.vector.tensor_scalar(
                s2[:, :f], tt[:, :f], -0.5, None, op0=AluOpType.add
            )
            # previous tile's Ln comes after this tile's s2 in the ACT stream
            if pend is not None:
                emit_ln(pend)
            # y2 = (p - 0.5) * s2
            y2 = work_pool.tile([P, MAXF], mybir.dt.float32, tag="y2")
            nc.vector.scalar_tensor_tensor(
                y2[:, :f], pt[:, :f], 0.5, s2[:, :f],
                op0=AluOpType.subtract, op1=AluOpType.mult,
            )
            # count mask: (p - 1) > s2  <=>  (t == 0) & (p > 0.5)
            cmask = out_sc.tile([P, MAXF], mybir.dt.float32, tag="c")
            nc.vector.scalar_tensor_tensor(
                cmask[:, :f], pt[:, :f], 1.0, s2[:, :f],
                op0=AluOpType.subtract, op1=AluOpType.is_gt,
                accum_out=acc_cnt[:, i : i + 1],
            )
            pend = (y2, f, i)
        emit_ln(pend)
        nc.sync.dma_start(out_dram[:, :NTILES], acc_cnt[:])
        nc.sync.dma_start(out_dram[:, NTILES:], acc_ln[:])
    nc.compile()
    return nc


def kernel(inputs: np.ndarray, targets: np.ndarray) -> np.ndarray:
    global last_results
    inputs = np.asarray(inputs, dtype=np.float32)
    targets = np.asarray(targets, dtype=np.int32)
    assert inputs.shape == (N,) and targets.shape == (N,)

    nc = _build()
    in_maps = []
    for c in range(NCORES):
        sl = slice(c * PER, (c + 1) * PER)
        in_maps.append(
            {
                "inputs": np.ascontiguousarray(inputs[sl]).reshape(P, FREE),
                "targets": np.ascontiguousarray(targets[sl]).reshape(P, FREE),
            }
        )
    res = run_bass_kernel_spmd(nc, in_maps, list(range(NCORES)))
    last_results = res

    cnt = 0.0
    lnsum = 0.0
    for r in res.results:
        part = np.asarray(r["partials"], dtype=np.float64)
        cnt += part[:, :NTILES].sum()
        lnsum += part[:, NTILES:].sum()
    loss = -(lnsum / N) * (1.0 + 0.1 * cnt)
    return np.asarray(loss, dtype=np.float32)

